# revision 48
# baseline (speedup 1.0000x reference)
"""Trainium2 Bass kernel for BERTIdealEmissionRateCompressionModule.

reference math (teacher path):
    head_mean = attentions.mean(axis=2)          # [L, B, S, S]
    prod      = prod_L head_mean                 # [B, S, S]
    y_soft    = -prod[:, 0, :]                   # [B, S]   <- only CLS row used!
    y_hard    = rank(y_soft with y[0]=min-1) < k # [B, S] bool, stable ranking

Only attentions[:, :, :, 0, :] (L*H*S floats per batch row) is live data.
Sharding: pure data parallel over batch B=8 -> one batch row per NeuronCore.
The host also pre-transposes each core's slice to token-major [S, L*H] so the
device needs no layout work on the input.

Per-core device pipeline (attT [512, 144] f32, 144 = L*H):
  1. DMA attT -> SBUF as [128, 4*144] (token p, free = (seg t, l, h))
  2. one reduce over h -> sums [128, 4*12]; fused *1/144 into the first
     pairwise multiply; pairwise tree over l -> prodall [128, 4]
     (prodall[p, t] = prod of head-means for token j = 128t + p)
  3. y_soft columns = -prodall (packed output cols 0:4)
  4. psel = prod with psel[0] = 1.0 (1.0 > any product of softmax means, so
     CLS always ranks first; degenerate inputs hit the host tie-fallback)
  5. bc[p, i] = psel[i]: replicate each psel column along free on DVE, then
     PE-transpose segments into one PSUM tile (identity const, 1 small DMA)
  6. strict rank[j] = #{i: psel[i] > psel[j]} via fused is_gt + row-sum
     (tensor_scalar accum_out), one op per 128-token segment
  7. y_hard = rank < k -> packed output cols 4:8; single DMA out
Host: reorders columns to rows, casts mask to bool; if any y_soft row has
duplicate values (exact ties -- impossible for real attention products), the
mask is recomputed on host with the reference's stable double-argsort.
"""

import functools

import numpy as np

L, B, H, S = 12, 8, 12, 512
LH = L * H  # 144
N_CORES = 8
SEG = 128
N_SEG = S // SEG  # 4


@functools.lru_cache(maxsize=4)
def _build(k: int):
    import concourse.bass as bass
    import concourse.mybir as mybir
    from concourse.tile import TileContext
    from concourse.vector_clock import ScopedClock

    class TileContextSplitDrain(TileContext):
        """This walrus codegen fits a single embedded sync wait per
        instruction; Tile's kernel-tail drain aggregates one wait per live
        semaphore onto one Drain. Split it into a chain of single-wait
        drains on the sync queue (same semantics: all waits complete
        before the end-of-kernel barrier)."""

        def _drain_and_barrier(self, tick_clock, wait_clock):
            nc = self.nc
            drain_inst = nc.sync.drain()
            wait_clock.add_sem_waits(
                drain_inst.ins, ScopedClock({None: tick_clock.global_clock})
            )
            si = drain_inst.ins.sync_info
            if si is not None and len(si.on_wait) > 1:
                waits = list(si.on_wait)
                ups = list(si.on_update)
                drain_inst.ins.sync_info = mybir.SyncInfo(
                    on_wait=[waits[0]], on_update=[])
                for w in waits[1:-1]:
                    d = nc.sync.drain()
                    d.ins.sync_info = mybir.SyncInfo(on_wait=[w], on_update=[])
                d = nc.sync.drain()
                d.ins.sync_info = mybir.SyncInfo(
                    on_wait=[waits[-1]], on_update=ups)
            nc.all_engine_barrier()
            assert self.sems is not None
            popped = nc._tile_sem_poison_stack.pop()
            assert popped is self._sem_poison
            nc.clear_and_free_semaphores(list(self.sems.allocated().values()))
            nc.all_engine_barrier()

    f32 = mybir.dt.float32
    Alu = mybir.AluOpType
    X = mybir.AxisListType.X

    f16 = mybir.dt.float16

    nc = bass.Bass()
    # host pre-packs to the exact SBUF image: [128, (seg t, l, h)]
    attT = nc.declare_dram_parameter("attT", [SEG, N_SEG * LH], f32,
                                     isOutput=False)
    # y_soft in column form: ys[p, t] = y_soft[128*t + p]; y_hard as a row
    y_soft_o = nc.dram_tensor("y_soft", [SEG, N_SEG], f32,
                              kind="ExternalOutput")
    y_hard_o = nc.dram_tensor("y_hard", [1, S], f32, kind="ExternalOutput")

    pack_d = nc.inline_tensor(np.eye(128, dtype=np.float32), "cpack")
    ones16_d = nc.inline_tensor(np.ones((128, 1), dtype=np.float16), "ones16")

    with TileContextSplitDrain(nc) as tc:
        with (
            tc.tile_pool(name="const", bufs=1) as cpool,
            tc.tile_pool(name="inp", bufs=1) as ipool,
            tc.tile_pool(name="work", bufs=2) as wpool,
            tc.tile_pool(name="rowbuf", bufs=1) as rpool,
            tc.tile_pool(name="cmp", bufs=N_SEG) as mpool,
            tc.tile_pool(name="pbc", bufs=1, space="PSUM") as pbc_pool,
            tc.tile_pool(name="prank", bufs=1, space="PSUM") as prank_pool,
            tc.tile_pool(name="pdum", bufs=1, space="PSUM") as pdum_pool,
        ):
            # input halves issued from different engines (SP + ACT) so they
            # dispatch and transfer in parallel; identity const second on ACT
            # (it is off the critical path)
            HALF = N_SEG * LH // 2
            at = ipool.tile([128, N_SEG * LH], f32, tag="at")
            nc.sync.dma_start(at[:, 0:HALF], attT[:, 0:HALF])
            nc.scalar.dma_start(at[:, HALF:], attT[:, HALF:])
            cpack = cpool.tile([128, 128], f32, tag="cpack")
            nc.scalar.dma_start(cpack[:], pack_d[:])
            id128 = cpack[:, 0:128]
            ones16 = cpool.tile([128, 1], f16, tag="ones16")
            nc.scalar.dma_start(ones16[:], ones16_d[:])

            # dummy transposes: make the PE take each const-DMA wait alone
            # (this walrus codegen fits a single sync wait per compute inst)
            # and warm the PE pipe during otherwise-idle time.
            pdum = pdum_pool.tile([128, 128], f32, tag="pdum")
            nc.tensor.transpose(pdum[:], id128, id128)
            pdum16 = pdum_pool.tile([1, 1], f16, tag="pdum16")
            nc.tensor.transpose(pdum16[:], ones16[0:1, 0:1], ones16[0:1, 0:1])

            # head sums over h for all segments: sums[p, (t, l)]; two halves
            # so each can start as soon as its DMA ring completes
            sums = wpool.tile([128, N_SEG * L], f32, tag="sums")
            nc.vector.tensor_reduce(
                sums[:, 0:N_SEG * L // 2],
                at[:, 0:HALF].rearrange("p (t l h) -> p t l h", l=L, h=H),
                axis=X, op=Alu.add)
            nc.vector.tensor_reduce(
                sums[:, N_SEG * L // 2:],
                at[:, HALF:].rearrange("p (t l h) -> p t l h", l=L, h=H),
                axis=X, op=Alu.add)

            # product over layers, 1/12 mean scales folded pairwise:
            # p6 = (s_l / 144) * s_{l+6} == (s_l/12) * (s_{l+6}/12) up to 1ulp
            sv = sums[:].rearrange("p (t l) -> p t l", l=L)
            p6 = wpool.tile([128, N_SEG * 6], f32, tag="p6")
            p6v = p6[:].rearrange("p (t l) -> p t l", l=6)
            nc.vector.scalar_tensor_tensor(
                p6v, sv[:, :, 0:6], float(np.float32(1.0 / 144.0)),
                sv[:, :, 6:12], op0=Alu.mult, op1=Alu.mult)
            p3 = wpool.tile([128, N_SEG * 3], f32, tag="p3")
            p3v = p3[:].rearrange("p (t l) -> p t l", l=3)
            nc.vector.tensor_tensor(p3v, p6v[:, :, 0:3], p6v[:, :, 3:6],
                                    op=Alu.mult)
            p1 = wpool.tile([128, N_SEG], f32, tag="p1")
            p1v = p1[:].rearrange("p (t l) -> p t l", l=1)
            nc.vector.tensor_tensor(p1v, p3v[:, :, 0:1], p3v[:, :, 1:2],
                                    op=Alu.mult)
            prodall = wpool.tile([128, N_SEG], f32, tag="prodall")
            nc.vector.tensor_tensor(
                prodall[:].rearrange("p (t l) -> p t l", l=1), p1v,
                p3v[:, :, 2:3], op=Alu.mult)

            # y_soft columns = -prod, shipped out early (overlaps ranking)
            out_s = rpool.tile([128, N_SEG], f32, tag="out")
            nc.vector.tensor_scalar_mul(out_s[:], prodall[:], -1.0)
            nc.sync.dma_start(y_soft_o[:], out_s[:])

            # CLS sentinel applied in place after y_soft has read prodall:
            # 1.0 > any product of softmax means, so CLS always ranks first
            # (degenerate inputs hit the host tie-fallback)
            cols = [prodall[:, t:t + 1] for t in range(N_SEG)]

            # bc[p, i] = psel[i]: replicate columns along free, PE-transpose.
            # Segments 1..3 don't depend on the CLS memset, so they go first;
            # the memset and segment 0 follow, keeping the PE fed earlier.
            psum_bc = pbc_pool.tile([128, S], f32, tag="bc")
            for t in (1, 2, 3):
                rep = mpool.tile([128, SEG], f32, tag="rep")
                nc.vector.tensor_copy(rep[:], cols[t].broadcast_to([128, SEG]))
                nc.tensor.transpose(
                    psum_bc[:, t * SEG:(t + 1) * SEG], rep[:], id128)
            nc.vector.memset(prodall[0:1, 0:1], 1.0)
            rep = mpool.tile([128, SEG], f32, tag="rep")
            nc.vector.tensor_copy(rep[:], cols[0].broadcast_to([128, SEG]))
            nc.tensor.transpose(psum_bc[:, 0:SEG], rep[:], id128)

            # strict rank[j] = #{i: psel[i] > psel[j]}; partition p = i-within-
            # segment (i = 128t+p), free f = j.  The 0/1 compare matrices are
            # written in fp16 (exact for 0/1) and summed over i by full-rate
            # fp16 PE matmuls accumulating into PSUM, overlapping the DVE.
            psum_rank = prank_pool.tile([1, S], f32, tag="rank")
            for t in range(N_SEG):
                gt = mpool.tile([128, S], f16, tag="gt")
                nc.vector.tensor_scalar(
                    gt[:], psum_bc[:], cols[t], None, op0=Alu.is_lt)
                nc.tensor.matmul(
                    psum_rank[:], ones16[:], gt[:],
                    start=(t == 0), stop=(t == N_SEG - 1))

            mask = rpool.tile([1, S], f32, tag="mask")
            nc.vector.tensor_scalar(
                mask[:], psum_rank[:], float(k), None, op0=Alu.is_lt)
            nc.sync.dma_start(y_hard_o[:], mask[:])

    return nc


LAST_RESULT = None  # BassKernelResults of the most recent run (for profiling)


def _ensure_ntff_hook():
    """bass_utils hard-imports antenv.axon_hooks when tracing is requested;
    this container's antenv lacks it. Provide it (with a working hook when
    the axon .so supports NRT profiling)."""
    import sys
    import types

    try:
        import antenv.axon_hooks  # noqa: F401

        return
    except ImportError:
        pass
    mod = types.ModuleType("antenv.axon_hooks")
    state = [None]
    mod.set_axon_ntff_profile_hook = lambda h: state.__setitem__(0, h)
    mod.get_axon_ntff_profile_hook = lambda: state[0]
    try:
        from trn_agent_boot.trn_boot import _ntff_profile_via_ctypes

        state[0] = _ntff_profile_via_ctypes("/opt/axon/libaxon_pjrt.so")
    except Exception:
        pass
    try:
        import antenv

        antenv.axon_hooks = mod
    except ImportError:
        pass
    sys.modules["antenv.axon_hooks"] = mod


def _run(attT_all: np.ndarray, k: int):
    global LAST_RESULT
    _ensure_ntff_hook()
    from concourse.bass_utils import run_bass_kernel_spmd

    nc = _build(k)
    in_maps = [{"attT": attT_all[b]} for b in range(B)]
    LAST_RESULT = run_bass_kernel_spmd(nc, in_maps, list(range(N_CORES)))
    res = LAST_RESULT.results
    y_soft = np.stack([res[b]["y_soft"].T.reshape(S) for b in range(B)])
    y_hard = np.stack([res[b]["y_hard"][0] for b in range(B)]) > 0.5
    if any(np.unique(y_soft[b]).size != S for b in range(B)):
        # exact duplicate values: strict rank != stable rank; replicate the
        # reference's stable double-argsort on host (f32, global min)
        y = y_soft.copy()
        y[:, 0] = np.float32(y_soft.min() - np.float32(1.0))
        order = np.argsort(y, axis=-1, kind="stable")
        rank = np.argsort(order, axis=-1, kind="stable")
        y_hard = rank < k
    return y_hard, y_soft


def kernel(attentions, embedding_sequence, compression_rate):
    att = np.asarray(attentions)
    seq_len = int(np.asarray(embedding_sequence).shape[1])
    k = max(int(seq_len * (1.0 - float(np.asarray(compression_rate)))), 1)
    # live data: CLS attention row only, packed per batch to the SBUF image
    # [128, (seg, l, h)]: row p, col (t*144 + l*12 + h) = att[l, b, h, 0, 128t+p]
    attT_all = np.ascontiguousarray(
        att[:, :, :, 0, :].transpose(1, 3, 0, 2)     # [B, S, L, H]
        .reshape(B, N_SEG, SEG, LH).transpose(0, 2, 1, 3)
        .reshape(B, SEG, N_SEG * LH),
        dtype=np.float32)
    y_hard, y_soft = _run(attT_all, k)
    return y_hard, y_soft


# revision 50
# speedup vs baseline: 1.0994x; 1.0994x over previous
"""Trainium2 Bass kernel for BERTIdealEmissionRateCompressionModule.

reference math (teacher path):
    head_mean = attentions.mean(axis=2)          # [L, B, S, S]
    prod      = prod_L head_mean                 # [B, S, S]
    y_soft    = -prod[:, 0, :]                   # [B, S]   <- only CLS row used!
    y_hard    = rank(y_soft with y[0]=min-1) < k # [B, S] bool, stable ranking

Only attentions[:, :, :, 0, :] (L*H*S floats per batch row) is live data.
Sharding: pure data parallel over batch B=8 -> one batch row per NeuronCore.
The host also pre-transposes each core's slice to token-major [S, L*H] so the
device needs no layout work on the input.

Per-core device pipeline (attT [512, 144] f32, 144 = L*H):
  1. DMA attT -> SBUF as [128, 4*144] (token p, free = (seg t, l, h))
  2. one reduce over h -> sums [128, 4*12]; fused *1/144 into the first
     pairwise multiply; pairwise tree over l -> prodall [128, 4]
     (prodall[p, t] = prod of head-means for token j = 128t + p)
  3. y_soft columns = -prodall (packed output cols 0:4)
  4. psel = prod with psel[0] = 1.0 (1.0 > any product of softmax means, so
     CLS always ranks first; degenerate inputs hit the host tie-fallback)
  5. bc[p, i] = psel[i]: replicate each psel column along free on DVE, then
     PE-transpose segments into one PSUM tile (identity const, 1 small DMA)
  6. strict rank[j] = #{i: psel[i] > psel[j]} via fused is_gt + row-sum
     (tensor_scalar accum_out), one op per 128-token segment
  7. y_hard = rank < k -> packed output cols 4:8; single DMA out
Host: reorders columns to rows, casts mask to bool; if any y_soft row has
duplicate values (exact ties -- impossible for real attention products), the
mask is recomputed on host with the reference's stable double-argsort.
"""

import functools

import numpy as np

L, B, H, S = 12, 8, 12, 512
LH = L * H  # 144
N_CORES = 8
SEG = 128
N_SEG = S // SEG  # 4


@functools.lru_cache(maxsize=4)
def _build(k: int):
    import concourse.bass as bass
    import concourse.mybir as mybir
    from concourse.tile import TileContext
    from concourse.vector_clock import ScopedClock

    class TileContextSplitDrain(TileContext):
        """This walrus codegen fits a single embedded sync wait per
        instruction; Tile's kernel-tail drain aggregates one wait per live
        semaphore onto one Drain. Split it into a chain of single-wait
        drains on the sync queue (same semantics: all waits complete
        before the end-of-kernel barrier)."""

        def _drain_and_barrier(self, tick_clock, wait_clock):
            nc = self.nc
            drain_inst = nc.sync.drain()
            wait_clock.add_sem_waits(
                drain_inst.ins, ScopedClock({None: tick_clock.global_clock})
            )
            si = drain_inst.ins.sync_info
            if si is not None and len(si.on_wait) > 1:
                waits = list(si.on_wait)
                ups = list(si.on_update)
                drain_inst.ins.sync_info = mybir.SyncInfo(
                    on_wait=[waits[0]], on_update=[])
                for w in waits[1:-1]:
                    d = nc.sync.drain()
                    d.ins.sync_info = mybir.SyncInfo(on_wait=[w], on_update=[])
                d = nc.sync.drain()
                d.ins.sync_info = mybir.SyncInfo(
                    on_wait=[waits[-1]], on_update=ups)
            nc.all_engine_barrier()
            assert self.sems is not None
            popped = nc._tile_sem_poison_stack.pop()
            assert popped is self._sem_poison
            nc.clear_and_free_semaphores(list(self.sems.allocated().values()))
            # no trailing all_engine_barrier: nothing reads semaphores after
            # the clear, and NEFF completion already requires every engine
            # and DMA queue to finish (executions are serialized), so the
            # final alignment barrier only adds tail latency.

    f32 = mybir.dt.float32
    Alu = mybir.AluOpType
    X = mybir.AxisListType.X

    nc = bass.Bass()
    # host pre-packs to the exact SBUF image: [128, (seg t, l, h)]
    attT = nc.declare_dram_parameter("attT", [SEG, N_SEG * LH], f32,
                                     isOutput=False)
    # packed column-form output: cols 0:4 = y_soft, cols 4:8 = y_hard mask,
    # out[p, c] for token j = 128*(c%4) + p
    y_out = nc.dram_tensor("y_out", [SEG, 2 * N_SEG], f32,
                           kind="ExternalOutput")

    pack_d = nc.inline_tensor(np.eye(128, dtype=np.float32), "cpack")

    with TileContextSplitDrain(nc) as tc:
        with (
            tc.tile_pool(name="const", bufs=1) as cpool,
            tc.tile_pool(name="inp", bufs=1) as ipool,
            tc.tile_pool(name="work", bufs=2) as wpool,
            tc.tile_pool(name="rowbuf", bufs=1) as rpool,
            tc.tile_pool(name="cmp", bufs=N_SEG) as mpool,
            tc.tile_pool(name="pbc", bufs=1, space="PSUM") as pbc_pool,
            tc.tile_pool(name="pdum", bufs=1, space="PSUM") as pdum_pool,
        ):
            # input halves issued from different engines (SP + ACT) so they
            # dispatch and transfer in parallel; identity const second on ACT
            # (it is off the critical path)
            HALF = N_SEG * LH // 2
            at = ipool.tile([128, N_SEG * LH], f32, tag="at")
            nc.sync.dma_start(at[:, 0:HALF], attT[:, 0:HALF])
            nc.scalar.dma_start(at[:, HALF:], attT[:, HALF:])
            cpack = cpool.tile([128, 128], f32, tag="cpack")
            nc.scalar.dma_start(cpack[:], pack_d[:])
            id128 = cpack[:, 0:128]

            # dummy full-width transpose: makes the PE take the const-DMA
            # wait alone (this walrus codegen fits a single sync wait per
            # compute inst) AND warms the PE pipe during otherwise-idle time
            # so the first real transpose runs at steady-state cost.
            pdum = pdum_pool.tile([128, 128], f32, tag="pdum")
            nc.tensor.transpose(pdum[:], id128, id128)

            # head sums over h for all segments: sums[p, (t, l)]; two halves
            # so each can start as soon as its DMA ring completes
            sums = wpool.tile([128, N_SEG * L], f32, tag="sums")
            nc.vector.tensor_reduce(
                sums[:, 0:N_SEG * L // 2],
                at[:, 0:HALF].rearrange("p (t l h) -> p t l h", l=L, h=H),
                axis=X, op=Alu.add)
            nc.vector.tensor_reduce(
                sums[:, N_SEG * L // 2:],
                at[:, HALF:].rearrange("p (t l h) -> p t l h", l=L, h=H),
                axis=X, op=Alu.add)

            # product over layers, 1/12 mean scales folded pairwise:
            # p6 = (s_l / 144) * s_{l+6} == (s_l/12) * (s_{l+6}/12) up to 1ulp
            sv = sums[:].rearrange("p (t l) -> p t l", l=L)
            p6 = wpool.tile([128, N_SEG * 6], f32, tag="p6")
            p6v = p6[:].rearrange("p (t l) -> p t l", l=6)
            nc.vector.scalar_tensor_tensor(
                p6v, sv[:, :, 0:6], float(np.float32(1.0 / 144.0)),
                sv[:, :, 6:12], op0=Alu.mult, op1=Alu.mult)
            p3 = wpool.tile([128, N_SEG * 3], f32, tag="p3")
            p3v = p3[:].rearrange("p (t l) -> p t l", l=3)
            nc.vector.tensor_tensor(p3v, p6v[:, :, 0:3], p6v[:, :, 3:6],
                                    op=Alu.mult)
            p1 = wpool.tile([128, N_SEG], f32, tag="p1")
            p1v = p1[:].rearrange("p (t l) -> p t l", l=1)
            nc.vector.tensor_tensor(p1v, p3v[:, :, 0:1], p3v[:, :, 1:2],
                                    op=Alu.mult)
            prodall = wpool.tile([128, N_SEG], f32, tag="prodall")
            nc.vector.tensor_tensor(
                prodall[:].rearrange("p (t l) -> p t l", l=1), p1v,
                p3v[:, :, 2:3], op=Alu.mult)

            # packed output: y_soft columns = -prod
            out_s = rpool.tile([128, 2 * N_SEG], f32, tag="out")
            nc.vector.tensor_scalar_mul(out_s[:, 0:N_SEG], prodall[:], -1.0)

            # CLS sentinel applied in place after y_soft has read prodall:
            # 1.0 > any product of softmax means, so CLS always ranks first
            # (degenerate inputs hit the host tie-fallback)
            cols = [prodall[:, t:t + 1] for t in range(N_SEG)]

            # bc[p, i] = psel[i]: replicate columns along free, PE-transpose.
            # Segments 1..3 don't depend on the CLS memset, so they go first;
            # the memset and segment 0 follow, keeping the PE fed earlier.
            psum_bc = pbc_pool.tile([128, S], f32, tag="bc")
            for t in (1, 2, 3):
                rep = mpool.tile([128, SEG], f32, tag="rep")
                nc.vector.tensor_copy(rep[:], cols[t].broadcast_to([128, SEG]))
                nc.tensor.transpose(
                    psum_bc[:, t * SEG:(t + 1) * SEG], rep[:], id128)
            nc.vector.memset(prodall[0:1, 0:1], 1.0)
            rep = mpool.tile([128, SEG], f32, tag="rep")
            nc.vector.tensor_copy(rep[:], cols[0].broadcast_to([128, SEG]))
            nc.tensor.transpose(psum_bc[:, 0:SEG], rep[:], id128)

            # strict rank[j] = #{i: psel[i] > psel[j]}; partition p = j-within-
            # segment (j = 128t+p), free f = i
            ranks = wpool.tile([128, N_SEG], f32, tag="ranks")
            for t in range(N_SEG):
                gt = mpool.tile([128, S], f32, tag="gt")
                nc.vector.tensor_scalar(
                    gt[:], psum_bc[:], cols[t], None, op0=Alu.is_gt,
                    op1=Alu.add, accum_out=ranks[:, t:t + 1])
            nc.vector.tensor_scalar(
                out_s[:, N_SEG:], ranks[:], float(k), None, op0=Alu.is_lt)

            nc.sync.dma_start(y_out[:], out_s[:])

    return nc


LAST_RESULT = None  # BassKernelResults of the most recent run (for profiling)


def _ensure_ntff_hook():
    """bass_utils hard-imports antenv.axon_hooks when tracing is requested;
    this container's antenv lacks it. Provide it (with a working hook when
    the axon .so supports NRT profiling)."""
    import sys
    import types

    try:
        import antenv.axon_hooks  # noqa: F401

        return
    except ImportError:
        pass
    mod = types.ModuleType("antenv.axon_hooks")
    state = [None]
    mod.set_axon_ntff_profile_hook = lambda h: state.__setitem__(0, h)
    mod.get_axon_ntff_profile_hook = lambda: state[0]
    try:
        from trn_agent_boot.trn_boot import _ntff_profile_via_ctypes

        state[0] = _ntff_profile_via_ctypes("/opt/axon/libaxon_pjrt.so")
    except Exception:
        pass
    try:
        import antenv

        antenv.axon_hooks = mod
    except ImportError:
        pass
    sys.modules["antenv.axon_hooks"] = mod


def _run(attT_all: np.ndarray, k: int):
    global LAST_RESULT
    _ensure_ntff_hook()
    from concourse.bass_utils import run_bass_kernel_spmd

    nc = _build(k)
    in_maps = [{"attT": attT_all[b]} for b in range(B)]
    LAST_RESULT = run_bass_kernel_spmd(nc, in_maps, list(range(N_CORES)))
    res = LAST_RESULT.results
    y_soft = np.stack([res[b]["y_out"][:, 0:N_SEG].T.reshape(S)
                       for b in range(B)])
    y_hard = np.stack([res[b]["y_out"][:, N_SEG:].T.reshape(S)
                       for b in range(B)]) > 0.5
    if any(np.unique(y_soft[b]).size != S for b in range(B)):
        # exact duplicate values: strict rank != stable rank; replicate the
        # reference's stable double-argsort on host (f32, global min)
        y = y_soft.copy()
        y[:, 0] = np.float32(y_soft.min() - np.float32(1.0))
        order = np.argsort(y, axis=-1, kind="stable")
        rank = np.argsort(order, axis=-1, kind="stable")
        y_hard = rank < k
    return y_hard, y_soft


def kernel(attentions, embedding_sequence, compression_rate):
    att = np.asarray(attentions)
    seq_len = int(np.asarray(embedding_sequence).shape[1])
    k = max(int(seq_len * (1.0 - float(np.asarray(compression_rate)))), 1)
    # live data: CLS attention row only, packed per batch to the SBUF image
    # [128, (seg, l, h)]: row p, col (t*144 + l*12 + h) = att[l, b, h, 0, 128t+p]
    attT_all = np.ascontiguousarray(
        att[:, :, :, 0, :].transpose(1, 3, 0, 2)     # [B, S, L, H]
        .reshape(B, N_SEG, SEG, LH).transpose(0, 2, 1, 3)
        .reshape(B, SEG, N_SEG * LH),
        dtype=np.float32)
    y_hard, y_soft = _run(attT_all, k)
    return y_hard, y_soft
